# revision 20
# baseline (speedup 1.0000x reference)
"""Trainium2 Bass kernel for nn_GCNHierarchicalClassifier.

Hierarchical GCN: 3 GCN layers on a 30k-node/960k-edge graph -> mean-pool to
3k clusters -> softmax classifier (output x0) -> 3 GCN layers on a pooled
3k-node/48k-edge graph -> mean-pool to 16 graphs -> softmax classifier (xf).

Strategy (8 NeuronCores):
- Nodes sharded contiguously across cores; edges partitioned by dst owner.
- Segment-sum (message aggregation) via one-hot scatter-matmul: edges packed
  into 128-edge chunks per 128-dst tile; gathered messages G [128e, F] via
  dma_gather from an fp16 node table in HBM; S [128e, 128dst] built on DVE as
  (iota == dst_local) * norm; PSUM accumulates S^T @ G per dst tile.
- GCN layer l: g = h @ (W diag(s_bn)) locally (GEMM, fp16), AllGather g into a
  full table, aggregate, then BN bias (+ReLU) fused into ScalarE activation
  after a PE transpose back to feature-major for the next GEMM.
- Layer 1 aggregates first (input x is replicated): AGG=Ax then GEMM.
- All index preprocessing (edge sort/pack, gcn norms, pooling counts, bpool)
  is host-side numpy; the instruction schedule is identical on all 8 cores
  (per-tile chunk counts are maxed across cores, padded with zero-weight
  edges), so one SPMD NEFF serves all cores with per-core input data.
"""

import numpy as np
from contextlib import ExitStack

import concourse.bacc as bacc
import concourse.mybir as mybir
import concourse.tile as tile
from concourse.bass_utils import run_bass_kernel_spmd

dt = mybir.dt
AF = mybir.ActivationFunctionType
ALU = mybir.AluOpType

# Problem dims (hardcoded per spec)
N0, F0, H, C, N1, B = 30000, 128, 256, 16, 3000, 16
EPS = 1e-5
NCORES = 8
NPC = N0 // NCORES          # 3750 nodes/core
NPC_PAD = 3840              # 30 tiles of 128
NT0 = NPC_PAD // 128        # 30
CPC = N1 // NCORES          # 375 clusters/core
CPC_PAD = 384               # 3 tiles of 128
NT1 = CPC_PAD // 128        # 3
GB = 18                     # gather chunks per dma_gather batch
NGP = (NCORES * CPC_PAD) // 128  # 24 graph-pool chunks

f16, f32, i16 = dt.float16, dt.float32, dt.int16


# ---------------------------------------------------------------- host side

def _gcn_norm(ei, n):
    """PyG gcn_norm with self loops. Returns (src, dst, norm) incl self loops."""
    sl = np.arange(n, dtype=np.int64)
    src = np.concatenate([ei[0].astype(np.int64), sl])
    dst = np.concatenate([ei[1].astype(np.int64), sl])
    deg = np.bincount(dst, minlength=n).astype(np.float32)
    dinv = np.zeros(n, np.float32)
    nz = deg > 0
    dinv[nz] = (1.0 / np.sqrt(deg[nz])).astype(np.float32)
    norm = (dinv[src] * dinv[dst]).astype(np.float32)
    return src, dst, norm


def _pack_edges(src_rows, dst, w, npc_dst, ntiles):
    """Pack weighted edges by dst into per-core / per-dst-tile 128-edge chunks.

    src_rows: table row index of the message source (already padded-mapped).
    Returns dict with cpt [ntiles], nch, and per-core packed flat arrays
    srcp [NC, nch*128] int32, dinp, nrmp (float32), plus SBUF-layout arrays.
    """
    owner = dst // npc_dst
    dloc = dst - owner * npc_dst
    tl = dloc >> 7
    din = dloc & 127
    gt = owner * ntiles + tl
    order = np.argsort(gt, kind="stable")
    gt_s = gt[order]
    counts = np.bincount(gt, minlength=NCORES * ntiles)
    cpt = (counts.reshape(NCORES, ntiles) + 127) // 128
    cpt = np.maximum(cpt.max(axis=0), 1).astype(np.int64)   # [ntiles]
    nch = int(cpt.sum())
    starts = np.zeros(NCORES * ntiles + 1, np.int64)
    starts[1:] = np.cumsum(counts)
    j = np.arange(len(gt_s), dtype=np.int64) - starts[gt_s]
    tile_off = np.zeros(ntiles, np.int64)
    tile_off[1:] = np.cumsum(cpt[:-1]) * 128
    core_of = gt_s // ntiles
    t_of = gt_s % ntiles
    pos = tile_off[t_of] + j

    srcp = np.zeros((NCORES, nch * 128), np.int32)
    dinp = np.zeros((NCORES, nch * 128), np.float32)
    nrmp = np.zeros((NCORES, nch * 128), np.float32)
    srcp[core_of, pos] = src_rows[order]
    dinp[core_of, pos] = din[order]
    nrmp[core_of, pos] = w[order]

    cols = nch * 8
    idx_l = np.tile(
        srcp.astype(np.int16).reshape(NCORES, cols, 16).transpose(0, 2, 1),
        (1, 8, 1),
    )  # [NC, 128, cols]
    dstL = dinp.astype(np.float16).reshape(NCORES, nch, 128).transpose(0, 2, 1)
    nrmL = nrmp.astype(np.float16).reshape(NCORES, nch, 128).transpose(0, 2, 1)
    return dict(cpt=cpt, nch=nch, srcp=srcp, dinp=dinp, nrmp=nrmp,
                idx=np.ascontiguousarray(idx_l),
                dst=np.ascontiguousarray(dstL),
                nrm=np.ascontiguousarray(nrmL))


def preprocess(inputs):
    """All host-side index/constant preparation. Returns meta dict."""
    ei = np.asarray(inputs["edge_index"])
    pool1 = np.asarray(inputs["pool1"]).astype(np.int64)
    batch = np.asarray(inputs["batch"]).astype(np.float32)
    ei1 = np.asarray(inputs["edge_index_pool1"])

    m = {}
    # --- phase A edges ---
    srcA, dstA, nrmA = _gcn_norm(ei, N0)
    srcA_pad = (srcA // NPC) * NPC_PAD + srcA % NPC
    m["A"] = _pack_edges(srcA_pad, dstA, nrmA, NPC, NT0)

    # --- pooling nodes -> clusters ---
    n_all = np.arange(N0, dtype=np.int64)
    cnt = np.bincount(pool1, minlength=N1).astype(np.float32)
    wpool = 1.0 / np.maximum(cnt, 1.0)
    m["P"] = _pack_edges((n_all // NPC) * NPC_PAD + n_all % NPC,
                         pool1, wpool[pool1].astype(np.float32), CPC, NT1)

    # --- phase B edges ---
    srcB, dstB, nrmB = _gcn_norm(ei1, N1)
    srcB_pad = (srcB // CPC) * CPC_PAD + srcB % CPC
    m["B"] = _pack_edges(srcB_pad, dstB, nrmB, CPC, NT1)

    # --- bpool / graph pooling ---
    sumb = np.bincount(pool1, weights=batch, minlength=N1).astype(np.float32)
    bpool = np.round(sumb / np.maximum(cnt, 1.0)).astype(np.int64)
    cnt_b = np.bincount(bpool, minlength=B).astype(np.float32)
    wgp_c = 1.0 / np.maximum(cnt_b, 1.0)
    # padded cluster row r -> (bpool, w) or (0, 0) for pads
    r = np.arange(NCORES * CPC_PAD, dtype=np.int64)
    is_real = (r % CPC_PAD) < CPC
    cl = (r // CPC_PAD) * CPC + np.minimum(r % CPC_PAD, CPC - 1)
    dst_gp = np.where(is_real, bpool[cl], 0).astype(np.float16)
    w_gp = np.where(is_real, wgp_c[bpool[cl]], 0.0).astype(np.float16)
    m["dstGP"] = np.ascontiguousarray(dst_gp.reshape(NGP, 128).T)  # [128, NGP]
    m["wGP"] = np.ascontiguousarray(w_gp.reshape(NGP, 128).T)
    m["bpool"] = bpool
    m["cnt"] = cnt
    m["cnt_b"] = cnt_b

    # --- BN folds + weights (fp16) ---
    def fold(g, be, mm, v, b):
        s = (np.asarray(g) / np.sqrt(np.asarray(v) + EPS)).astype(np.float32)
        t = ((np.asarray(b) - np.asarray(mm)) * s + np.asarray(be)).astype(np.float32)
        return s, t

    g0, be0 = np.asarray(inputs["g0"]), np.asarray(inputs["be0"])
    m0_, v0 = np.asarray(inputs["m0"]), np.asarray(inputs["v0"])
    b0 = np.asarray(inputs["b0"])
    g1_, be1 = np.asarray(inputs["g1"]), np.asarray(inputs["be1"])
    m1_, v1 = np.asarray(inputs["m1"]), np.asarray(inputs["v1"])
    b1 = np.asarray(inputs["b1"])
    W_in0 = np.asarray(inputs["W_in0"]); W_h0 = np.asarray(inputs["W_h0"])
    W_in1 = np.asarray(inputs["W_in1"]); W_h1 = np.asarray(inputs["W_h1"])

    sA, tA = zip(*[fold(g0[i], be0[i], m0_[i], v0[i], b0[i]) for i in range(3)])
    sB, tB = zip(*[fold(g1_[i], be1[i], m1_[i], v1[i], b1[i]) for i in range(3)])

    m["w1p"] = (W_in0 * sA[0]).astype(np.float16)                  # [128,256]
    m["w2p"] = (W_h0[0] * sA[1]).astype(np.float16)                # [256,256]
    m["w3p"] = (W_h0[1] * sA[2]).astype(np.float16)
    wb1 = np.zeros((128, H), np.float32)
    wb1[:C + 1] = W_in1 * sB[0]
    m["wb1p"] = wb1.astype(np.float16)                             # [128,256]
    m["wb2p"] = (W_h1[0] * sB[1]).astype(np.float16)
    m["wb3p"] = (W_h1[1] * sB[2]).astype(np.float16)
    # bias vectors as [128, 2] per-partition columns (feature-major)
    m["tv1"] = np.ascontiguousarray(tA[0].reshape(2, 128).T)       # [128,2] f32
    m["tv2"] = np.ascontiguousarray(tA[1].reshape(2, 128).T)
    m["t3b"] = np.tile(tA[2], (128, 1)).astype(np.float32)         # [128,256]
    m["tvb1"] = np.ascontiguousarray(tB[0].reshape(2, 128).T)
    m["tvb2"] = np.ascontiguousarray(tB[1].reshape(2, 128).T)
    m["tb3b"] = np.tile(tB[2], (128, 1)).astype(np.float32)
    m["lw0"] = np.asarray(inputs["linW0"]).astype(np.float16)      # [256,16]
    m["lb0"] = np.asarray(inputs["linb0"]).reshape(1, C).astype(np.float16)
    m["lw1"] = np.asarray(inputs["linW1"]).astype(np.float16)
    m["lb1"] = np.asarray(inputs["linb1"]).reshape(1, C).astype(np.float16)

    # x table fp16, padded rows
    x = np.asarray(inputs["x"]).astype(np.float16)
    xt = np.zeros((NCORES * NPC_PAD, F0), np.float16)
    for k in range(NCORES):
        xt[k * NPC_PAD:k * NPC_PAD + NPC] = x[k * NPC:(k + 1) * NPC]
    m["x_tab"] = xt
    xp1 = np.asarray(inputs["x_pool1"]).astype(np.float32)
    xp = np.zeros((NCORES, CPC_PAD, 1), np.float32)
    for k in range(NCORES):
        xp[k, :CPC] = xp1[k * CPC:(k + 1) * CPC]
    m["xp1s"] = xp

    m["iota"] = np.tile(np.arange(128, dtype=np.float16), (128, 1))
    m["ident"] = np.eye(128, dtype=np.float16)
    m["ones_row"] = np.ones((1, 128), np.float16)
    return m


# ---------------------------------------------------------------- bass build

def build(meta, upto="full"):
    STAGES = ["l1", "l2", "l3", "pool", "b1", "b2", "b3", "full"]
    stage_n = STAGES.index(upto)
    nchA, cptA = meta["A"]["nch"], meta["A"]["cpt"]
    nchP, cptP = meta["P"]["nch"], meta["P"]["cpt"]
    nchB, cptB = meta["B"]["nch"], meta["B"]["cpt"]

    nc = bacc.Bacc("TRN2", target_bir_lowering=False, debug=False,
                   num_devices=NCORES, dynamic_dma_scratch_size=131072)

    # ---- dram I/O ----
    din = {}
    def ein(name, shape, d):
        din[name] = nc.dram_tensor(name, list(shape), d, kind="ExternalInput")
        return din[name]

    x_tab = ein("x_tab", (NCORES * NPC_PAD, F0), f16)
    eidxA = ein("eidxA", (128, nchA * 8), i16)
    dstA_d = ein("dstA", (128, nchA), f16)
    nrmA_d = ein("nrmA", (128, nchA), f16)
    eidxP = ein("eidxP", (128, nchP * 8), i16)
    dstP_d = ein("dstP", (128, nchP), f16)
    nrmP_d = ein("nrmP", (128, nchP), f16)
    eidxB = ein("eidxB", (128, nchB * 8), i16)
    dstB_d = ein("dstB", (128, nchB), f16)
    nrmB_d = ein("nrmB", (128, nchB), f16)
    dstGP_d = ein("dstGP", (128, NGP), f16)
    wGP_d = ein("wGP", (128, NGP), f16)
    xp1s_d = ein("xp1s", (CPC_PAD, 1), f32)
    w1p_d = ein("w1p", (128, H), f16)
    w2p_d = ein("w2p", (H, H), f16)
    w3p_d = ein("w3p", (H, H), f16)
    wb1p_d = ein("wb1p", (128, H), f16)
    wb2p_d = ein("wb2p", (H, H), f16)
    wb3p_d = ein("wb3p", (H, H), f16)
    lw0_d = ein("lw0", (H, C), f16)
    lb0_d = ein("lb0", (1, C), f16)
    lw1_d = ein("lw1", (H, C), f16)
    lb1_d = ein("lb1", (1, C), f16)
    tv1_d = ein("tv1", (128, 2), f32)
    tv2_d = ein("tv2", (128, 2), f32)
    t3b_d = ein("t3b", (128, H), f32)
    tvb1_d = ein("tvb1", (128, 2), f32)
    tvb2_d = ein("tvb2", (128, 2), f32)
    tb3b_d = ein("tb3b", (128, H), f32)
    iota_d = ein("iota", (128, 128), f16)
    ident_d = ein("ident", (128, 128), f16)
    ones_d = ein("ones_row", (1, 128), f16)

    x0_out = nc.dram_tensor("x0_out", [CPC, C], f32, kind="ExternalOutput")
    xf_out = nc.dram_tensor("xf_out", [B, C], f32, kind="ExternalOutput")

    # AG bounce (Internal local) + tables (Shared)
    def agpair(name, rows_l, rows_g, width):
        a = nc.dram_tensor(name + "_in", [rows_l, width], f16)
        b_ = nc.dram_tensor(name + "_tab", [rows_g, width], f16,
                            addr_space="Shared")
        return a, b_

    g2_in, g2_tab = agpair("g2", NPC_PAD, NCORES * NPC_PAD, H)
    g3_in, g3_tab = agpair("g3", NPC_PAD, NCORES * NPC_PAD, H)
    x3_in, x3_tab = agpair("x3", NPC_PAD, NCORES * NPC_PAD, H)
    gp1_in, gp1_tab = agpair("gp1", CPC_PAD, NCORES * CPC_PAD, H)
    gp2_in, gp2_tab = agpair("gp2", CPC_PAD, NCORES * CPC_PAD, H)
    gp3_in, gp3_tab = agpair("gp3", CPC_PAD, NCORES * CPC_PAD, H)
    hp3_in, hp3_tab = agpair("hp3", CPC_PAD, NCORES * CPC_PAD, H)

    RG = [list(range(NCORES))]

    with tile.TileContext(nc) as tc, ExitStack() as ctx:
        cpool = ctx.enter_context(tc.tile_pool(name="consts", bufs=1))
        wk = ctx.enter_context(tc.tile_pool(name="work", bufs=2))
        gpool = ctx.enter_context(tc.tile_pool(name="gather", bufs=2))
        spool = ctx.enter_context(tc.tile_pool(name="onehot", bufs=2))
        htp = ctx.enter_context(tc.tile_pool(name="ht", bufs=1))
        pA = ctx.enter_context(tc.tile_pool(name="pagg", bufs=2, space="PSUM"))
        pG = ctx.enter_context(tc.tile_pool(name="pgemm", bufs=2, space="PSUM"))
        pT = ctx.enter_context(tc.tile_pool(name="ptp", bufs=2, space="PSUM"))

        def load(d, shape, dd, name):
            t = cpool.tile(list(shape), dd, tag=name)
            ap = d if isinstance(d, bacc.bass.AP) else d[:]
            nc.sync.dma_start(t[:], ap)
            return t

        iota_t = load(iota_d, (128, 128), f16, "iota")
        ident_t = load(ident_d, (128, 128), f16, "ident")
        ones_t = load(ones_d, (1, 128), f16, "ones")
        idxA_t = load(eidxA, (128, nchA * 8), i16, "idxA")
        dA_t = load(dstA_d, (128, nchA), f16, "dA")
        nA_t = load(nrmA_d, (128, nchA), f16, "nA")
        idxP_t = load(eidxP, (128, nchP * 8), i16, "idxP")
        dP_t = load(dstP_d, (128, nchP), f16, "dP")
        nP_t = load(nrmP_d, (128, nchP), f16, "nP")
        idxB_t = load(eidxB, (128, nchB * 8), i16, "idxB")
        dB_t = load(dstB_d, (128, nchB), f16, "dB")
        nB_t = load(nrmB_d, (128, nchB), f16, "nB")
        dGP_t = load(dstGP_d, (128, NGP), f16, "dGP")
        wGP_t = load(wGP_d, (128, NGP), f16, "wGP")
        w1_t = load(w1p_d, (128, H), f16, "w1")
        w2_t = [load(w2p_d[k * 128:(k + 1) * 128, :], (128, H), f16, f"w2_{k}")
                for k in range(2)]
        w3_t = [load(w3p_d[k * 128:(k + 1) * 128, :], (128, H), f16, f"w3_{k}")
                for k in range(2)]
        wb1_t = load(wb1p_d, (128, H), f16, "wb1")
        wb2_t = [load(wb2p_d[k * 128:(k + 1) * 128, :], (128, H), f16, f"wb2_{k}")
                 for k in range(2)]
        wb3_t = [load(wb3p_d[k * 128:(k + 1) * 128, :], (128, H), f16, f"wb3_{k}")
                 for k in range(2)]
        lw0_t = [load(lw0_d[k * 128:(k + 1) * 128, :], (128, C), f16, f"lw0_{k}")
                 for k in range(2)]
        lw1_t = [load(lw1_d[k * 128:(k + 1) * 128, :], (128, C), f16, f"lw1_{k}")
                 for k in range(2)]
        lb0_t = load(lb0_d, (1, C), f16, "lb0")
        lb1_t = load(lb1_d, (1, C), f16, "lb1")
        tv1_t = load(tv1_d, (128, 2), f32, "tv1")
        tv2_t = load(tv2_d, (128, 2), f32, "tv2")
        t3b_t = load(t3b_d, (128, H), f32, "t3b")
        tvb1_t = load(tvb1_d, (128, 2), f32, "tvb1")
        tvb2_t = load(tvb2_d, (128, 2), f32, "tvb2")
        tb3b_t = load(tb3b_d, (128, H), f32, "tb3b")
        # ---------- generic aggregation ----------
        def aggregate(tag, nch, cpt, elem, idx_t, d_t, n_t, table, out_cb,
                      swap_out=None):
            """Chunks of 128 edges: gather G, build S, matmul into PSUM.
            out_cb(t, agg_psum) with agg [128, elem] f32 per dst tile.
            If swap_out is not None -> produce transposed accumulation:
            swap_out(t, [psum_k ...]) with psum_k [128, elem_out] f32,
            computed as G_chunk[:, k]T-contract (for pooling/classifier)."""
            nbat = (nch + GB - 1) // GB
            Gb, Sb = [None] * nbat, [None] * nbat

            def ensure(b_):
                if Gb[b_] is None:
                    nb = min(GB, nch - b_ * GB)
                    g_t = gpool.tile([128, nb, elem], f16, tag="G" + str(elem))
                    nc.gpsimd.dma_gather(
                        g_t[:], table[:], idx_t[:, b_ * GB * 8:(b_ * GB + nb) * 8],
                        num_idxs=nb * 128, num_idxs_reg=nb * 128, elem_size=elem,
                        single_packet=False)
                    s_t = spool.tile([128, nb, 128], f16, tag="S")
                    db = d_t[:, b_ * GB:b_ * GB + nb, None].to_broadcast([128, nb, 128])
                    ib = iota_t[:, None, :].to_broadcast([128, nb, 128])
                    nb_b = n_t[:, b_ * GB:b_ * GB + nb, None].to_broadcast([128, nb, 128])
                    nc.vector.tensor_tensor(s_t[:], db, ib, ALU.is_equal)
                    nc.vector.tensor_tensor(s_t[:], s_t[:], nb_b, ALU.mult)
                    Gb[b_], Sb[b_] = g_t, s_t
                return Gb[b_], Sb[b_]

            c = 0
            for t, n_tile in enumerate(cpt):
                n_tile = int(n_tile)
                if swap_out is None:
                    agg = pA.tile([128, elem], f32, tag="agg", padded_shape=[128, 512])
                    for j in range(n_tile):
                        b_, slot = divmod(c, GB)
                        g_t, s_t = ensure(b_)
                        nc.tensor.matmul(agg[:], s_t[:, slot, :], g_t[:, slot, :],
                                         start=(j == 0), stop=(j == n_tile - 1))
                        c += 1
                    out_cb(t, agg)
                else:
                    nk = elem // 128
                    accT = [pG.tile([128, 128], f32, tag="gemm", name=f"accT{k_}", padded_shape=[128, 512])
                            for k_ in range(nk)]
                    for j in range(n_tile):
                        b_, slot = divmod(c, GB)
                        g_t, s_t = ensure(b_)
                        for k in range(nk):
                            nc.tensor.matmul(
                                accT[k][:], g_t[:, slot, k * 128:(k + 1) * 128],
                                s_t[:, slot, :],
                                start=(j == 0), stop=(j == n_tile - 1))
                        c += 1
                    swap_out(t, accT)

        # ---------- feature-major BN(+ReLU) callback ----------
        def cb_transposed(hT, tv_t, relu, elem):
            func = AF.Relu if relu else AF.Identity
            def cb(t, agg):
                aggs = wk.tile([128, elem], f16, tag="aggs")
                nc.vector.tensor_copy(aggs[:], agg[:])
                for k in range(elem // 128):
                    tp = pT.tile([128, 128], f16, tag="tp", padded_shape=[128, 1024])
                    nc.tensor.transpose(tp[:], aggs[:, k * 128:(k + 1) * 128],
                                        ident_t[:])
                    nc.scalar.activation(hT[:, k, t * 128:(t + 1) * 128], tp[:],
                                         func, bias=tv_t[:, k:k + 1], scale=1.0)
            return cb

        # ---------- L1: aggregate x first, then GEMM + BN + ReLU ----------
        h1T = htp.tile([128, 2, NPC_PAD], f16, tag="hTA")

        def cb_L1(t, agg):
            aggs = wk.tile([128, F0], f16, tag="aggs1")
            nc.vector.tensor_copy(aggs[:], agg[:])
            tp = pT.tile([128, 128], f16, tag="tp", padded_shape=[128, 1024])
            nc.tensor.transpose(tp[:], aggs[:], ident_t[:])
            aggT = wk.tile([128, 128], f16, tag="aggT1")
            nc.vector.tensor_copy(aggT[:], tp[:])
            g1 = pG.tile([128, H], f32, tag="gemm", padded_shape=[128, 512])
            nc.tensor.matmul(g1[:], aggT[:], w1_t[:], start=True, stop=True)
            g1s = wk.tile([128, H], f16, tag="gs")
            nc.vector.tensor_copy(g1s[:], g1[:])
            for k in range(2):
                tp2 = pT.tile([128, 128], f16, tag="tp", padded_shape=[128, 1024])
                nc.tensor.transpose(tp2[:], g1s[:, k * 128:(k + 1) * 128],
                                    ident_t[:])
                nc.scalar.activation(h1T[:, k, t * 128:(t + 1) * 128], tp2[:],
                                     AF.Relu, bias=tv1_t[:, k:k + 1], scale=1.0)

        aggregate("L1", nchA, cptA, F0, idxA_t, dA_t, nA_t, x_tab, cb_L1)

        # ---------- GEMM pass helper ----------
        def gemm_pass(hT, w_tiles, bounce, ntiles):
            for i in range(ntiles):
                g = pG.tile([128, H], f32, tag="gemm", padded_shape=[128, 512])
                for k in range(len(w_tiles)):
                    nc.tensor.matmul(g[:], hT[:, k, i * 128:(i + 1) * 128],
                                     w_tiles[k][:],
                                     start=(k == 0), stop=(k == len(w_tiles) - 1))
                gs = wk.tile([128, H], f16, tag="gs")
                nc.vector.tensor_copy(gs[:], g[:])
                nc.sync.dma_start(bounce[i * 128:(i + 1) * 128, :], gs[:])

        def allgather(src, dst_tab):
            nc.gpsimd.collective_compute(
                "AllGather", ALU.bypass, replica_groups=RG,
                ins=[src[:].opt()], outs=[dst_tab[:].opt()])

        # ---------- L2 ----------
        gemm_pass(h1T, w2_t, g2_in, NT0)
        allgather(g2_in, g2_tab)
        h2T = htp.tile([128, 2, NPC_PAD], f16, tag="hTA")
        aggregate("L2", nchA, cptA, H, idxA_t, dA_t, nA_t, g2_tab,
                  cb_transposed(h2T, tv2_t, True, H))

        # ---------- L3 (node-major output, no relu) ----------
        gemm_pass(h2T, w3_t, g3_in, NT0)
        allgather(g3_in, g3_tab)

        def cb_L3(t, agg):
            x3t = wk.tile([128, H], f16, tag="gs")
            nc.vector.tensor_tensor(x3t[:], agg[:], t3b_t[:], ALU.add)
            nc.sync.dma_start(x3_in[t * 128:(t + 1) * 128, :], x3t[:])

        aggregate("L3", nchA, cptA, H, idxA_t, dA_t, nA_t, g3_tab, cb_L3)
        allgather(x3_in, x3_tab)

        # ---------- pooling to clusters + cluster classifier + Z ----------
        zT_tiles = []

        def pool_out(ct, accT):
            # accT: [2] psum [128 f, 128 c] f32 = pooledT for cluster tile ct
            xpT_s = []
            for k in range(2):
                tt = wk.tile([128, 128], f16, tag="xpTs")
                nc.vector.tensor_copy(tt[:], accT[k][:])
                xpT_s.append(tt)
            lg = pG.tile([128, C], f32, tag="lg", padded_shape=[128, 512])
            nc.tensor.matmul(lg[:], xpT_s[0][:], lw0_t[0][:], start=True, stop=False)
            nc.tensor.matmul(lg[:], xpT_s[1][:], lw0_t[1][:], start=False, stop=False)
            nc.tensor.matmul(lg[:], ones_t[:, 0:128], lb0_t[:], start=False, stop=True)
            negmax = wk.tile([128, 1], f32, tag="nm")
            nc.vector.tensor_reduce(negmax[:], lg[:], axis=mybir.AxisListType.X,
                                    op=ALU.max, negate=True)
            expt = wk.tile([128, C], f32, tag="expt")
            nc.scalar.activation(expt[:], lg[:], AF.Exp, bias=negmax[:], scale=1.0)
            ssum = wk.tile([128, 1], f32, tag="ss")
            nc.vector.reduce_sum(ssum[:], expt[:], axis=mybir.AxisListType.X)
            rinv = wk.tile([128, 1], f32, tag="ri")
            nc.vector.reciprocal(rinv[:], ssum[:])
            x0t = wk.tile([128, C], f32, tag="x0t")
            nc.vector.tensor_scalar_mul(x0t[:], expt[:], rinv[:])
            sz = min(128, CPC - ct * 128)
            nc.sync.dma_start(x0_out[ct * 128:ct * 128 + sz, :], x0t[:sz, :])
            # Z tile: [x0 | x_pool1 | 0...]
            z_t = wk.tile([128, 128], f16, tag="z")
            nc.gpsimd.memset(z_t[:], 0.0)
            nc.vector.tensor_copy(z_t[:, 0:C], x0t[:])
            xpcol = wk.tile([128, 1], f32, tag="xpc")
            nc.sync.dma_start(xpcol[:], xp1s_d[ct * 128:(ct + 1) * 128, :])
            nc.vector.tensor_copy(z_t[:, C:C + 1], xpcol[:])
            # transpose for p1 GEMM
            ztp = pT.tile([128, 128], f16, tag="tp", padded_shape=[128, 1024])
            nc.tensor.transpose(ztp[:], z_t[:], ident_t[:])
            zT = wk.tile([128, 128], f16, tag="zT" + str(ct))
            nc.vector.tensor_copy(zT[:], ztp[:])
            zT_tiles.append(zT)

        aggregate("P", nchP, cptP, H, idxP_t, dP_t, nP_t, x3_tab, None,
                  swap_out=pool_out)

        # ---------- phase B layer p1 ----------
        for ct in range(NT1):
            g = pG.tile([128, H], f32, tag="gemm", padded_shape=[128, 512])
            nc.tensor.matmul(g[:], zT_tiles[ct][:], wb1_t[:], start=True, stop=True)
            gs = wk.tile([128, H], f16, tag="gs")
            nc.vector.tensor_copy(gs[:], g[:])
            nc.sync.dma_start(gp1_in[ct * 128:(ct + 1) * 128, :], gs[:])
        allgather(gp1_in, gp1_tab)
        hB1T = htp.tile([128, 2, CPC_PAD], f16, tag="hTB")
        aggregate("B1", nchB, cptB, H, idxB_t, dB_t, nB_t, gp1_tab,
                  cb_transposed(hB1T, tvb1_t, True, H))

        # ---------- p2 ----------
        gemm_pass(hB1T, wb2_t, gp2_in, NT1)
        allgather(gp2_in, gp2_tab)
        hB2T = htp.tile([128, 2, CPC_PAD], f16, tag="hTB")
        aggregate("B2", nchB, cptB, H, idxB_t, dB_t, nB_t, gp2_tab,
                  cb_transposed(hB2T, tvb2_t, True, H))

        # ---------- p3 (node-major, no relu) ----------
        gemm_pass(hB2T, wb3_t, gp3_in, NT1)
        allgather(gp3_in, gp3_tab)

        def cb_B3(t, agg):
            hp = wk.tile([128, H], f16, tag="gs")
            nc.vector.tensor_tensor(hp[:], agg[:], tb3b_t[:], ALU.add)
            nc.sync.dma_start(hp3_in[t * 128:(t + 1) * 128, :], hp[:])

        aggregate("B3", nchB, cptB, H, idxB_t, dB_t, nB_t, gp3_tab, cb_B3)
        allgather(hp3_in, hp3_tab)

        # ---------- graph pooling + final classifier ----------
        sgp = spool.tile([128, NGP, 128], f16, tag="S")
        dgb = dGP_t[:, :, None].to_broadcast([128, NGP, 128])
        igb = iota_t[:, None, :].to_broadcast([128, NGP, 128])
        wgb = wGP_t[:, :, None].to_broadcast([128, NGP, 128])
        nc.vector.tensor_tensor(sgp[:], dgb, igb, ALU.is_equal)
        nc.vector.tensor_tensor(sgp[:], sgp[:], wgb, ALU.mult)
        xfT = [pG.tile([128, C], f32, tag="lg", name=f"xfT{k_}", padded_shape=[128, 512])
               for k_ in range(2)]
        # (xfT shares the "lg" psum tag: 2 bufs hold exactly the 2 accumulators)
        for ch in range(NGP):
            hp = wk.tile([128, H], f16, tag="hpch")
            nc.sync.dma_start(hp[:], hp3_tab[ch * 128:(ch + 1) * 128, :])
            for k in range(2):
                nc.tensor.matmul(xfT[k][:], hp[:, k * 128:(k + 1) * 128],
                                 sgp[:, ch, 0:C],
                                 start=(ch == 0), stop=(ch == NGP - 1))
        xfT_s = []
        for k in range(2):
            tt = wk.tile([128, C], f16, tag="xfTs")
            nc.vector.tensor_copy(tt[:], xfT[k][:])
            xfT_s.append(tt)
        lgf = pG.tile([B, C], f32, tag="lg", padded_shape=[128, 512])
        nc.tensor.matmul(lgf[:], xfT_s[0][:, 0:B], lw1_t[0][:], start=True, stop=False)
        nc.tensor.matmul(lgf[:], xfT_s[1][:, 0:B], lw1_t[1][:], start=False, stop=False)
        nc.tensor.matmul(lgf[:], ones_t[:, 0:B], lb1_t[:], start=False, stop=True)
        negmax = wk.tile([B, 1], f32, tag="nmf")
        nc.vector.tensor_reduce(negmax[:], lgf[:], axis=mybir.AxisListType.X,
                                op=ALU.max, negate=True)
        expt = wk.tile([B, C], f32, tag="exptf")
        nc.scalar.activation(expt[:], lgf[:], AF.Exp, bias=negmax[:], scale=1.0)
        ssum = wk.tile([B, 1], f32, tag="ssf")
        nc.vector.reduce_sum(ssum[:], expt[:], axis=mybir.AxisListType.X)
        rinv = wk.tile([B, 1], f32, tag="rif")
        nc.vector.reciprocal(rinv[:], ssum[:])
        xft = wk.tile([B, C], f32, tag="xft")
        nc.vector.tensor_scalar_mul(xft[:], expt[:], rinv[:])
        nc.sync.dma_start(xf_out[:], xft[:])

    nc.compile()
    return nc


def make_in_maps(meta):
    shared = dict(
        x_tab=meta["x_tab"], dstGP=meta["dstGP"], wGP=meta["wGP"],
        w1p=meta["w1p"], w2p=meta["w2p"], w3p=meta["w3p"],
        wb1p=meta["wb1p"], wb2p=meta["wb2p"], wb3p=meta["wb3p"],
        lw0=meta["lw0"], lb0=meta["lb0"], lw1=meta["lw1"], lb1=meta["lb1"],
        tv1=meta["tv1"], tv2=meta["tv2"], t3b=meta["t3b"],
        tvb1=meta["tvb1"], tvb2=meta["tvb2"], tb3b=meta["tb3b"],
        iota=meta["iota"], ident=meta["ident"], ones_row=meta["ones_row"],
    )
    maps = []
    for k in range(NCORES):
        m = dict(shared)
        m["eidxA"] = meta["A"]["idx"][k]
        m["dstA"] = meta["A"]["dst"][k]
        m["nrmA"] = meta["A"]["nrm"][k]
        m["eidxP"] = meta["P"]["idx"][k]
        m["dstP"] = meta["P"]["dst"][k]
        m["nrmP"] = meta["P"]["nrm"][k]
        m["eidxB"] = meta["B"]["idx"][k]
        m["dstB"] = meta["B"]["dst"][k]
        m["nrmB"] = meta["B"]["nrm"][k]
        m["xp1s"] = meta["xp1s"][k]
        maps.append(m)
    return maps


def run(meta, trace=False, upto="full", nc=None, **kw):
    if nc is None:
        nc = build(meta, upto=upto)
    res = run_bass_kernel_spmd(nc, make_in_maps(meta),
                               core_ids=list(range(NCORES)), trace=trace, **kw)
    x0 = np.concatenate([res.results[k]["x0_out"] for k in range(NCORES)], 0)
    xf = res.results[0]["xf_out"]
    return (x0.astype(np.float32), xf.astype(np.float32)), res, nc


def kernel(**inputs):
    meta = preprocess(inputs)
    (x0, xf), _, _ = run(meta, trace=False)
    return (x0, xf)
